# revision 8
# baseline (speedup 1.0000x reference)
"""Trainium2 Bass kernel for nn_ClEncoder (MLP encoder + InfoNCE-style loss).

Computation (reference):
    h  = relu(features @ W1 + b1)         # [40000, 2048] @ [2048, 256]
    h  = relu(h @ W2 + b2)                # [40000, 256] @ [256, 64]
    x1 = concat([feature_users, h]) @ W3 + b3   # [90000, 64]
    loss = info_nce(view0, view1)         # scalar from [64, 512, 64] views

Sharding (8 cores, data-parallel):
    - features rows 40000 -> 5000/core (padded to 5120)
    - feature_users rows 50000 -> 6250/core (padded to 6656)
    - view group axis G=64 -> 8 groups/core
    - weights replicated

Layout trick: all activations are kept transposed ([D, rows]) on device so
every matmul has its contraction dim on SBUF partitions with *natural*
weight layout and zero on-chip transposes.  The host pre-transposes the
per-core features/feature_users slices (cheap numpy prep) and transposes
the small [64, rows] outputs back.  The scalar loss reduction over per-group
sums (8x64 per core) is done on host from tiny per-core partials.
"""

import numpy as np

import concourse.bass as bass
import concourse.mybir as mybir
import concourse.tile as tile
from concourse.bass_utils import run_bass_kernel_spmd

F32 = mybir.dt.float32

# Problem dims (hardcoded per task contract).
FEAT = 2048
DLAT = 64
H1 = 256
NUM_USER = 50000
NUM_ITEM = 40000
G = 64
M = 512
EPS = 1e-12

NCORES = 8
IPC = NUM_ITEM // NCORES          # 5000 item rows per core
IPAD = 5120                        # = 10 * 512
UPC = NUM_USER // NCORES          # 6250 user rows per core
UPAD = 6656                        # = 13 * 512
GPC = G // NCORES                 # 8 groups per core
VROWS = GPC * M                   # 4096 view rows per core

NB_I = IPAD // 512                # item blocks
NB_U = UPAD // 512                # user blocks
KC = FEAT // 128                  # 16 contraction chunks for layer 1
NT_V = VROWS // 128               # 32 view tiles of 128 rows


def _split_multi_waits(nc):
    """Walrus's per-instruction codegen accepts only one semaphore wait on
    some instruction structs (e.g. LDWEIGHTS, TensorScalarPtr).  Tile can
    emit 2+ waits on one instruction; split the extras onto same-engine
    NoOps inserted immediately before, preserving per-engine order."""
    import bass_rust

    ctr = 0
    for blk in nc.m.functions[0].blocks:
        out = []
        changed = False
        for inst in blk.instructions:
            si = inst.sync_info
            waits = list(si.on_wait) if si is not None and si.on_wait else []
            if len(waits) > 1:
                changed = True
                for w in waits[:-1]:
                    ctr += 1
                    nop = bass_rust.InstNoOp(name=f"WSPLIT-{ctr}")
                    nop.engine = inst.engine
                    nop.sync_info = bass_rust.SyncInfo(
                        on_wait=[w], on_update=[]
                    )
                    out.append(nop)
                inst.sync_info = bass_rust.SyncInfo(
                    on_wait=[waits[-1]],
                    on_update=list(si.on_update) if si.on_update else [],
                )
            out.append(inst)
        if changed:
            blk.instructions = out
    return ctr


def _build_device_program():
    nc = bass.Bass(
        "TRN2",
        target_bir_lowering=False,
        debug=False,
        enable_asserts=False,
        num_devices=NCORES,
    )

    x_t = nc.dram_tensor("xT", [FEAT, IPAD], F32, kind="ExternalInput")
    u_t = nc.dram_tensor("uT", [DLAT, UPAD], F32, kind="ExternalInput")
    v0 = nc.dram_tensor("v0", [VROWS, DLAT], F32, kind="ExternalInput")
    v1 = nc.dram_tensor("v1", [VROWS, DLAT], F32, kind="ExternalInput")
    w1 = nc.dram_tensor("w1", [FEAT, H1], F32, kind="ExternalInput")
    w2 = nc.dram_tensor("w2", [H1, DLAT], F32, kind="ExternalInput")
    w3 = nc.dram_tensor("w3", [DLAT, DLAT], F32, kind="ExternalInput")
    b1 = nc.dram_tensor("b1", [128, 2], F32, kind="ExternalInput")
    b2 = nc.dram_tensor("b2", [DLAT, 1], F32, kind="ExternalInput")
    b3 = nc.dram_tensor("b3", [DLAT, 1], F32, kind="ExternalInput")

    oi_t = nc.dram_tensor("oiT", [DLAT, IPAD], F32, kind="ExternalOutput")
    ou_t = nc.dram_tensor("ouT", [DLAT, UPAD], F32, kind="ExternalOutput")
    u0_o = nc.dram_tensor("u0", [1, GPC * DLAT], F32, kind="ExternalOutput")
    u1_o = nc.dram_tensor("u1", [1, GPC * DLAT], F32, kind="ExternalOutput")

    Relu = mybir.ActivationFunctionType.Relu
    Ident = mybir.ActivationFunctionType.Identity

    with tile.TileContext(nc) as tc:
        with (
            tc.tile_pool(name="const", bufs=1) as cpool,
            tc.tile_pool(name="xstream", bufs=2) as xpool,
            tc.tile_pool(name="acts", bufs=2) as hpool,
            tc.tile_pool(name="users", bufs=2) as upool,
            tc.tile_pool(name="views", bufs=2) as vpool,
            tc.tile_pool(name="mm1", bufs=4, space="PSUM") as mpsum,
            tc.tile_pool(name="mm2", bufs=2, space="PSUM") as spsum,
            tc.tile_pool(name="vsum", bufs=2, space="PSUM") as vpsum,
        ):
            # ---- replicated constants ----
            w1s = cpool.tile([128, KC, H1], F32)
            nc.sync.dma_start(w1s, w1.ap().rearrange("(c p) m -> p c m", p=128))
            w2s = cpool.tile([128, 2, DLAT], F32)
            nc.sync.dma_start(w2s, w2.ap().rearrange("(c p) m -> p c m", p=128))
            w3s = cpool.tile([DLAT, DLAT], F32)
            nc.sync.dma_start(w3s, w3.ap())
            b1s = cpool.tile([128, 2], F32)
            nc.sync.dma_start(b1s, b1.ap())
            b2s = cpool.tile([DLAT, 1], F32)
            nc.sync.dma_start(b2s, b2.ap())
            b3s = cpool.tile([DLAT, 1], F32)
            nc.sync.dma_start(b3s, b3.ap())
            # ---- InfoNCE views: normalize rows, per-group sums ----
            for vin, vout in ((v0, u0_o), (v1, u1_o)):
                vs = vpool.tile([128, NT_V, DLAT], F32, tag="vs")
                nc.sync.dma_start(
                    vs, vin.ap().rearrange("(t p) e -> p t e", p=128)
                )
                sq = vpool.tile([128, NT_V, DLAT], F32, tag="sq")
                nc.vector.tensor_mul(sq, vs, vs)
                ss = vpool.tile([128, NT_V], F32, tag="ss")
                nc.vector.tensor_reduce(
                    ss, sq, axis=mybir.AxisListType.X, op=mybir.AluOpType.add
                )
                nn = vpool.tile([128, NT_V], F32, tag="nn")
                nc.scalar.sqrt(nn, ss)
                nc.vector.tensor_scalar_max(nn, nn, EPS)
                rn = vpool.tile([128, NT_V], F32, tag="rn")
                nc.vector.reciprocal(rn, nn)

                # Row-normalized group sums fold into the matmul: with
                # lhsT = rn[:, t] (one column of reciprocal norms) the PE
                # computes u_g = sum_p rn[p] * v[p, :], i.e. the sum of
                # L2-normalized rows — no explicit normalize pass needed.
                pu = vpsum.tile([1, GPC * DLAT], F32, tag="pu")
                for t in range(NT_V):
                    g = t // 4
                    nc.tensor.matmul(
                        pu[:, g * DLAT : (g + 1) * DLAT],
                        rn[:, t : t + 1],
                        vs[:, t, :],
                        start=(t % 4 == 0),
                        stop=(t % 4 == 3),
                    )
                us = vpool.tile([1, GPC * DLAT], F32, tag="us")
                nc.vector.tensor_copy(us, pu)
                nc.sync.dma_start(vout.ap(), us)

            # ---- items: 3-layer MLP on transposed activations ----
            xr = x_t.ap().rearrange("(c p) n -> p c n", p=128)  # [128, KC, IPAD]
            for b in range(NB_I):
                sl = bass.ds(b * 512, 512)
                xb = xpool.tile([128, KC, 512], F32, tag="xb")
                nc.sync.dma_start(xb, xr[:, :, sl])

                h1 = hpool.tile([128, 2, 512], F32, tag="h1")
                for m in range(2):
                    p1 = mpsum.tile([128, 512], F32, tag="p1")
                    for k in range(KC):
                        nc.tensor.matmul(
                            p1,
                            w1s[:, k, m * 128 : (m + 1) * 128],
                            xb[:, k, :],
                            start=(k == 0),
                            stop=(k == KC - 1),
                        )
                    nc.scalar.activation(
                        h1[:, m, :], p1, Relu, bias=b1s[:, m : m + 1]
                    )

                p2 = spsum.tile([DLAT, 512], F32, tag="ps")
                for k in range(2):
                    nc.tensor.matmul(
                        p2, w2s[:, k, :], h1[:, k, :], start=(k == 0), stop=(k == 1)
                    )
                h2 = hpool.tile([DLAT, 512], F32, tag="h2")
                nc.scalar.activation(h2, p2, Relu, bias=b2s[:, 0:1])

                p3 = spsum.tile([DLAT, 512], F32, tag="ps")
                nc.tensor.matmul(p3, w3s, h2, start=True, stop=True)
                o3 = hpool.tile([DLAT, 512], F32, tag="o3")
                nc.scalar.activation(o3, p3, Ident, bias=b3s[:, 0:1])
                nc.sync.dma_start(oi_t.ap()[:, sl], o3)

            # ---- users: single linear layer ----
            for b in range(NB_U):
                sl = bass.ds(b * 512, 512)
                ut = upool.tile([DLAT, 512], F32, tag="ut")
                nc.sync.dma_start(ut, u_t.ap()[:, sl])
                pu2 = spsum.tile([DLAT, 512], F32, tag="ps")
                nc.tensor.matmul(pu2, w3s, ut, start=True, stop=True)
                ot = upool.tile([DLAT, 512], F32, tag="ot")
                nc.scalar.activation(ot, pu2, Ident, bias=b3s[:, 0:1])
                nc.sync.dma_start(ou_t.ap()[:, sl], ot)

    _split_multi_waits(nc)
    return nc


_CACHE: dict = {}


def _get_program():
    if "nc" not in _CACHE:
        _CACHE["nc"] = _build_device_program()
    return _CACHE["nc"]


def _make_in_maps(features, feature_users, view0, view1, W1, b1, W2, b2, W3, b3):
    features = np.asarray(features, np.float32)
    feature_users = np.asarray(feature_users, np.float32)
    view0 = np.asarray(view0, np.float32)
    view1 = np.asarray(view1, np.float32)
    W1 = np.ascontiguousarray(np.asarray(W1, np.float32))
    W2 = np.ascontiguousarray(np.asarray(W2, np.float32))
    W3 = np.ascontiguousarray(np.asarray(W3, np.float32))
    b1d = np.ascontiguousarray(np.asarray(b1, np.float32).reshape(2, 128).T)
    b2d = np.ascontiguousarray(np.asarray(b2, np.float32).reshape(DLAT, 1))
    b3d = np.ascontiguousarray(np.asarray(b3, np.float32).reshape(DLAT, 1))

    in_maps = []
    for c in range(NCORES):
        xT = np.zeros((FEAT, IPAD), np.float32)
        xT[:, :IPC] = features[c * IPC : (c + 1) * IPC].T
        uT = np.zeros((DLAT, UPAD), np.float32)
        uT[:, :UPC] = feature_users[c * UPC : (c + 1) * UPC].T
        v0c = np.ascontiguousarray(
            view0[c * GPC : (c + 1) * GPC].reshape(VROWS, DLAT)
        )
        v1c = np.ascontiguousarray(
            view1[c * GPC : (c + 1) * GPC].reshape(VROWS, DLAT)
        )
        in_maps.append(
            {
                "xT": xT,
                "uT": uT,
                "v0": v0c,
                "v1": v1c,
                "w1": W1,
                "w2": W2,
                "w3": W3,
                "b1": b1d,
                "b2": b2d,
                "b3": b3d,
            }
        )
    return in_maps


def _combine_results(res):
    x1 = np.empty((NUM_USER + NUM_ITEM, DLAT), np.float32)
    U0 = np.empty((G, DLAT), np.float64)
    U1 = np.empty((G, DLAT), np.float64)
    for c in range(NCORES):
        x1[c * UPC : (c + 1) * UPC] = res[c]["ouT"][:, :UPC].T
        x1[NUM_USER + c * IPC : NUM_USER + (c + 1) * IPC] = res[c]["oiT"][:, :IPC].T
        U0[c * GPC : (c + 1) * GPC] = res[c]["u0"].reshape(GPC, DLAT)
        U1[c * GPC : (c + 1) * GPC] = res[c]["u1"].reshape(GPC, DLAT)

    pos = float((U0 * U0).sum() + (U0 * U1).sum())
    s0 = U0.sum(axis=0)
    s1 = U1.sum(axis=0)
    neg = float(s0 @ s0 + s0 @ s1) - pos
    loss = np.float32(neg / pos)
    return x1, loss


def kernel(features, feature_users, view0, view1, W1, b1, W2, b2, W3, b3):
    in_maps = _make_in_maps(
        features, feature_users, view0, view1, W1, b1, W2, b2, W3, b3
    )
    nc = _get_program()
    out = run_bass_kernel_spmd(nc, in_maps, list(range(NCORES)))
    return _combine_results(out.results)


# revision 10
# speedup vs baseline: 4.7147x; 4.7147x over previous
"""Trainium2 Bass kernel for nn_ClEncoder (MLP encoder + InfoNCE-style loss).

Computation (reference):
    h  = relu(features @ W1 + b1)         # [40000, 2048] @ [2048, 256]
    h  = relu(h @ W2 + b2)                # [40000, 256] @ [256, 64]
    x1 = concat([feature_users, h]) @ W3 + b3   # [90000, 64]
    loss = info_nce(view0, view1)         # scalar from [64, 512, 64] views

Sharding (8 cores, data-parallel):
    - features rows 40000 -> 5000/core (padded to 5120)
    - feature_users rows 50000 -> 6250/core (padded to 6656)
    - view group axis G=64 -> 8 groups/core
    - weights replicated

Layout trick: all activations are kept transposed ([D, rows]) on device so
every matmul has its contraction dim on SBUF partitions with *natural*
weight layout and zero on-chip transposes.  The host pre-transposes the
per-core features/feature_users slices (cheap numpy prep) and transposes
the small [64, rows] outputs back.  The scalar loss reduction over per-group
sums (8x64 per core) is done on host from tiny per-core partials.
"""

import numpy as np

import concourse.bass as bass
import concourse.mybir as mybir
import concourse.tile as tile
from concourse.bass_utils import run_bass_kernel_spmd

F32 = mybir.dt.float32

# Problem dims (hardcoded per task contract).
FEAT = 2048
DLAT = 64
H1 = 256
NUM_USER = 50000
NUM_ITEM = 40000
G = 64
M = 512
EPS = 1e-12

NCORES = 8
IPC = NUM_ITEM // NCORES          # 5000 item rows per core
IPAD = 5120                        # = 10 * 512
UPC = NUM_USER // NCORES          # 6250 user rows per core
UPAD = 6656                        # = 13 * 512
GPC = G // NCORES                 # 8 groups per core
VROWS = GPC * M                   # 4096 view rows per core

NB_I = IPAD // 512                # item blocks
NB_U = UPAD // 512                # user blocks
KC = FEAT // 128                  # 16 contraction chunks for layer 1
KH = KC // 2                      # half-block k chunks (DMA pipelining)
NT_V = VROWS // 128               # 32 view tiles of 128 rows


def _split_multi_waits(nc):
    """Walrus's per-instruction codegen accepts only one semaphore wait on
    some instruction structs (e.g. LDWEIGHTS, TensorScalarPtr).  Tile can
    emit 2+ waits on one instruction; split the extras onto same-engine
    NoOps inserted immediately before, preserving per-engine order."""
    import bass_rust

    ctr = 0
    for blk in nc.m.functions[0].blocks:
        out = []
        changed = False
        for inst in blk.instructions:
            si = inst.sync_info
            waits = list(si.on_wait) if si is not None and si.on_wait else []
            if len(waits) > 1:
                changed = True
                for w in waits[:-1]:
                    ctr += 1
                    nop = bass_rust.InstNoOp(name=f"WSPLIT-{ctr}")
                    nop.engine = inst.engine
                    nop.sync_info = bass_rust.SyncInfo(
                        on_wait=[w], on_update=[]
                    )
                    out.append(nop)
                inst.sync_info = bass_rust.SyncInfo(
                    on_wait=[waits[-1]],
                    on_update=list(si.on_update) if si.on_update else [],
                )
            out.append(inst)
        if changed:
            blk.instructions = out
    return ctr


def _emit_body(nc, pools, t):
    """Emit one full kernel body (weights load + views + items + users)."""
    cpool, xpool, hpool, upool, vpool, mpsum, spsum, vpsum = pools
    Relu = mybir.ActivationFunctionType.Relu
    Ident = mybir.ActivationFunctionType.Identity

    # ---- replicated constants ----
    w1s = cpool.tile([128, KC, H1], F32, tag="w1s")
    nc.sync.dma_start(w1s, t["w1"].ap().rearrange("(c p) m -> p c m", p=128))
    w2s = cpool.tile([128, 2, DLAT], F32, tag="w2s")
    nc.sync.dma_start(w2s, t["w2"].ap().rearrange("(c p) m -> p c m", p=128))
    w3s = cpool.tile([DLAT, DLAT], F32, tag="w3s")
    nc.sync.dma_start(w3s, t["w3"].ap())
    b1s = cpool.tile([128, 2], F32, tag="b1s")
    nc.sync.dma_start(b1s, t["b1"].ap())
    b2s = cpool.tile([DLAT, 1], F32, tag="b2s")
    nc.sync.dma_start(b2s, t["b2"].ap())
    b3s = cpool.tile([DLAT, 1], F32, tag="b3s")
    nc.sync.dma_start(b3s, t["b3"].ap())

    # ---- InfoNCE views: normalize rows, per-group sums ----
    for vin, vout in ((t["v0"], t["u0"]), (t["v1"], t["u1"])):
        vs = vpool.tile([128, NT_V, DLAT], F32, tag="vs")
        nc.sync.dma_start(vs, vin.ap().rearrange("(t p) e -> p t e", p=128))
        sq = vpool.tile([128, NT_V, DLAT], F32, tag="sq")
        nc.vector.tensor_mul(sq, vs, vs)
        ss = vpool.tile([128, NT_V], F32, tag="ss")
        nc.vector.tensor_reduce(
            ss, sq, axis=mybir.AxisListType.X, op=mybir.AluOpType.add
        )
        nn = vpool.tile([128, NT_V], F32, tag="nn")
        nc.scalar.sqrt(nn, ss)
        nc.vector.tensor_scalar_max(nn, nn, EPS)
        rn = vpool.tile([128, NT_V], F32, tag="rn")
        nc.vector.reciprocal(rn, nn)

        # Row-normalized group sums fold into the matmul: with
        # lhsT = rn[:, t] (one column of reciprocal norms) the PE
        # computes u_g = sum_p rn[p] * v[p, :], i.e. the sum of
        # L2-normalized rows — no explicit normalize pass needed.
        pu = vpsum.tile([1, GPC * DLAT], F32, tag="pu")
        for ti in range(NT_V):
            g = ti // 4
            nc.tensor.matmul(
                pu[:, g * DLAT : (g + 1) * DLAT],
                rn[:, ti : ti + 1],
                vs[:, ti, :],
                start=(ti % 4 == 0),
                stop=(ti % 4 == 3),
            )
        us = vpool.tile([1, GPC * DLAT], F32, tag="us")
        nc.vector.tensor_copy(us, pu)
        nc.sync.dma_start(vout.ap(), us)

    # ---- items: 3-layer MLP on transposed activations ----
    # Per 512-row block the [2048, 512] slice streams as two half DMAs
    # (k-chunks 0..7 / 8..15); matmuls run k-outer so the PE consumes
    # chunks in DMA arrival order.
    xr = t["xT"].ap().rearrange("(c p) n -> p c n", p=128)  # [128, KC, IPAD]
    for b in range(NB_I):
        sl = bass.ds(b * 512, 512)
        xh = []
        for h in range(2):
            xb = xpool.tile([128, KH, 512], F32, tag="xb")
            nc.sync.dma_start(xb, xr[:, h * KH : (h + 1) * KH, sl])
            xh.append(xb)

        p1a = mpsum.tile([128, 512], F32, tag="p1")
        p1b = mpsum.tile([128, 512], F32, tag="p1")
        p1 = [p1a, p1b]
        for k in range(KC):
            for m in range(2):
                nc.tensor.matmul(
                    p1[m],
                    w1s[:, k, m * 128 : (m + 1) * 128],
                    xh[k // KH][:, k % KH, :],
                    start=(k == 0),
                    stop=(k == KC - 1),
                )
        h1 = hpool.tile([128, 2, 512], F32, tag="h1")
        for m in range(2):
            nc.scalar.activation(h1[:, m, :], p1[m], Relu, bias=b1s[:, m : m + 1])

        p2 = spsum.tile([DLAT, 512], F32, tag="ps")
        for k in range(2):
            nc.tensor.matmul(
                p2, w2s[:, k, :], h1[:, k, :], start=(k == 0), stop=(k == 1)
            )
        h2 = hpool.tile([DLAT, 512], F32, tag="h2")
        nc.scalar.activation(h2, p2, Relu, bias=b2s[:, 0:1])

        p3 = spsum.tile([DLAT, 512], F32, tag="ps")
        nc.tensor.matmul(p3, w3s, h2, start=True, stop=True)
        o3 = hpool.tile([DLAT, 512], F32, tag="o3")
        nc.scalar.activation(o3, p3, Ident, bias=b3s[:, 0:1])
        nc.sync.dma_start(t["oiT"].ap()[:, sl], o3)

    # ---- users: single linear layer ----
    for b in range(NB_U):
        sl = bass.ds(b * 512, 512)
        ut = upool.tile([DLAT, 512], F32, tag="ut")
        nc.sync.dma_start(ut, t["uT"].ap()[:, sl])
        pu2 = spsum.tile([DLAT, 512], F32, tag="ps")
        nc.tensor.matmul(pu2, w3s, ut, start=True, stop=True)
        ot = upool.tile([DLAT, 512], F32, tag="ot")
        nc.scalar.activation(ot, pu2, Ident, bias=b3s[:, 0:1])
        nc.sync.dma_start(t["ouT"].ap()[:, sl], ot)


def _build_device_program(repeat=1):
    nc = bass.Bass(
        "TRN2",
        target_bir_lowering=False,
        debug=False,
        enable_asserts=False,
        num_devices=NCORES,
    )

    t = {
        "xT": nc.dram_tensor("xT", [FEAT, IPAD], F32, kind="ExternalInput"),
        "uT": nc.dram_tensor("uT", [DLAT, UPAD], F32, kind="ExternalInput"),
        "v0": nc.dram_tensor("v0", [VROWS, DLAT], F32, kind="ExternalInput"),
        "v1": nc.dram_tensor("v1", [VROWS, DLAT], F32, kind="ExternalInput"),
        "w1": nc.dram_tensor("w1", [FEAT, H1], F32, kind="ExternalInput"),
        "w2": nc.dram_tensor("w2", [H1, DLAT], F32, kind="ExternalInput"),
        "w3": nc.dram_tensor("w3", [DLAT, DLAT], F32, kind="ExternalInput"),
        "b1": nc.dram_tensor("b1", [128, 2], F32, kind="ExternalInput"),
        "b2": nc.dram_tensor("b2", [DLAT, 1], F32, kind="ExternalInput"),
        "b3": nc.dram_tensor("b3", [DLAT, 1], F32, kind="ExternalInput"),
        "oiT": nc.dram_tensor("oiT", [DLAT, IPAD], F32, kind="ExternalOutput"),
        "ouT": nc.dram_tensor("ouT", [DLAT, UPAD], F32, kind="ExternalOutput"),
        "u0": nc.dram_tensor("u0", [1, GPC * DLAT], F32, kind="ExternalOutput"),
        "u1": nc.dram_tensor("u1", [1, GPC * DLAT], F32, kind="ExternalOutput"),
    }

    with tile.TileContext(nc) as tc:
        with (
            tc.tile_pool(name="const", bufs=1) as cpool,
            tc.tile_pool(name="xstream", bufs=4) as xpool,
            tc.tile_pool(name="acts", bufs=2) as hpool,
            tc.tile_pool(name="users", bufs=2) as upool,
            tc.tile_pool(name="views", bufs=2) as vpool,
            tc.tile_pool(name="mm1", bufs=4, space="PSUM") as mpsum,
            tc.tile_pool(name="mm2", bufs=2, space="PSUM") as spsum,
            tc.tile_pool(name="vsum", bufs=2, space="PSUM") as vpsum,
        ):
            pools = (cpool, xpool, hpool, upool, vpool, mpsum, spsum, vpsum)
            for _ in range(repeat):
                _emit_body(nc, pools, t)

    _split_multi_waits(nc)
    return nc


_CACHE: dict = {}


def _get_program(repeat=1):
    key = ("nc", repeat)
    if key not in _CACHE:
        _CACHE[key] = _build_device_program(repeat)
    return _CACHE[key]


def _make_in_maps(features, feature_users, view0, view1, W1, b1, W2, b2, W3, b3):
    features = np.asarray(features, np.float32)
    feature_users = np.asarray(feature_users, np.float32)
    view0 = np.asarray(view0, np.float32)
    view1 = np.asarray(view1, np.float32)
    W1 = np.ascontiguousarray(np.asarray(W1, np.float32))
    W2 = np.ascontiguousarray(np.asarray(W2, np.float32))
    W3 = np.ascontiguousarray(np.asarray(W3, np.float32))
    b1d = np.ascontiguousarray(np.asarray(b1, np.float32).reshape(2, 128).T)
    b2d = np.ascontiguousarray(np.asarray(b2, np.float32).reshape(DLAT, 1))
    b3d = np.ascontiguousarray(np.asarray(b3, np.float32).reshape(DLAT, 1))

    in_maps = []
    for c in range(NCORES):
        xT = np.zeros((FEAT, IPAD), np.float32)
        xT[:, :IPC] = features[c * IPC : (c + 1) * IPC].T
        uT = np.zeros((DLAT, UPAD), np.float32)
        uT[:, :UPC] = feature_users[c * UPC : (c + 1) * UPC].T
        v0c = np.ascontiguousarray(
            view0[c * GPC : (c + 1) * GPC].reshape(VROWS, DLAT)
        )
        v1c = np.ascontiguousarray(
            view1[c * GPC : (c + 1) * GPC].reshape(VROWS, DLAT)
        )
        in_maps.append(
            {
                "xT": xT,
                "uT": uT,
                "v0": v0c,
                "v1": v1c,
                "w1": W1,
                "w2": W2,
                "w3": W3,
                "b1": b1d,
                "b2": b2d,
                "b3": b3d,
            }
        )
    return in_maps


def _combine_results(res):
    x1 = np.empty((NUM_USER + NUM_ITEM, DLAT), np.float32)
    U0 = np.empty((G, DLAT), np.float64)
    U1 = np.empty((G, DLAT), np.float64)
    for c in range(NCORES):
        x1[c * UPC : (c + 1) * UPC] = res[c]["ouT"][:, :UPC].T
        x1[NUM_USER + c * IPC : NUM_USER + (c + 1) * IPC] = res[c]["oiT"][:, :IPC].T
        U0[c * GPC : (c + 1) * GPC] = res[c]["u0"].reshape(GPC, DLAT)
        U1[c * GPC : (c + 1) * GPC] = res[c]["u1"].reshape(GPC, DLAT)

    pos = float((U0 * U0).sum() + (U0 * U1).sum())
    s0 = U0.sum(axis=0)
    s1 = U1.sum(axis=0)
    neg = float(s0 @ s0 + s0 @ s1) - pos
    loss = np.float32(neg / pos)
    return x1, loss


def kernel(features, feature_users, view0, view1, W1, b1, W2, b2, W3, b3):
    in_maps = _make_in_maps(
        features, feature_users, view0, view1, W1, b1, W2, b2, W3, b3
    )
    nc = _get_program()
    out = run_bass_kernel_spmd(nc, in_maps, list(range(NCORES)))
    return _combine_results(out.results)
